# revision 4
# baseline (speedup 1.0000x reference)
"""CrossAttention TRN2 kernel (head-parallel, fp16 operands, host-summed partials).

Problem (hardcoded shapes):
  x    [4, 2048, 1024], cond [4, 2048, 1024]
  Wq/Wk/Wv [1024, 1024], Wo [1024, 1024], bo [1024]
  out = softmax((x@Wq) 8 heads of 128 @ (cond@Wk)^T * 0.125) @ (cond@Wv) @ Wo + bo

Sharding: 8 cores = (batch b in 0..3) x (head-half hh in 0..1).
Each core computes Q/K/V projections and attention for ITS 4 heads over the
full 2048 queries, then a PARTIAL output projection (contraction over its 512
inner columns only), written as fp16 [2048, 1024]. The host sums the two
partials per batch and adds the bias — no duplicated projection work and no
on-chip collectives. Total matmul rows/core: 532k -> PE-bound at ~0.5ns/row.

Schedule notes (v2 — PE-dense restructure):
  - O projection accumulates over heads directly in PSUM at the END of the
    kernel (4 matmuls per [128,512] output chunk) instead of per-head DVE
    adds into an SBUF accumulator. This removes ~128 DVE tensor_tensor ops
    (~100us of DVE time) that made Vector the attention-phase bottleneck.
    The PSUM->SBUF output copies run on the Scalar engine (idle at the tail).
  - K/Q projections for heads 1-3 are emitted inside head-0/1's attention
    jt-loops, so the Tile scheduler fills PE gaps (while Act does exp) with
    projection matmuls instead of idling.
  - Scores kept transposed [j, i]; softmax denominator accumulated on DVE in
    fp16 (high priority), partition-reduced via ones-matmul, inverted with a
    single 1024-wide reciprocal_approx_fast.
  - PSUM: sc 2x[128,1024] (4 banks) + av 2x[128,512] (2) + pp 2x[128,512]
    (2) = 8 banks. O-proj accumulators and den broadcast reuse the sc tag.
"""
import numpy as np

import concourse.bacc as bacc
import concourse.tile as tile
from concourse import mybir
from concourse.bass_utils import run_bass_kernel_spmd

F32 = mybir.dt.float32
F16 = mybir.dt.float16
EXP = mybir.ActivationFunctionType.Exp

B, NQ, NK, D = 4, 2048, 2048, 1024
H, DH = 8, 128
SCALE = 64 ** -0.5
NCORES = 8
KT = D // 128                         # contraction tiles (8)
HL = 4                                # heads per core
JT = NK // 128                        # key tiles (16)
IT = NQ // 128                        # output row tiles (16)
MW = HL * DH                          # 512 inner columns per core


def build_nc():
    nc = bacc.Bacc()
    xT = nc.declare_dram_parameter("xT", [KT, 128, NQ], F16, isOutput=False)
    condT = nc.declare_dram_parameter("condT", [KT, 128, NK], F16, isOutput=False)
    wq = nc.declare_dram_parameter("wq", [KT, 128, MW], F16, isOutput=False)
    wk = nc.declare_dram_parameter("wk", [KT, 128, MW], F16, isOutput=False)
    wv = nc.declare_dram_parameter("wv", [KT, 128, MW], F16, isOutput=False)
    wo = nc.declare_dram_parameter("wo", [HL, 128, D], F16, isOutput=False)
    out = nc.declare_dram_parameter("out", [NQ, D], F16, isOutput=True)

    with tile.TileContext(nc) as tc:
        with (
            nc.allow_low_precision(reason="fp16 matmul operands are intended"),
            tc.tile_pool(name="const", bufs=1) as const,
            tc.tile_pool(name="big", bufs=1) as big,
            tc.tile_pool(name="expp", bufs=6) as expp,
            tc.tile_pool(name="den", bufs=2) as denp,
            tc.tile_pool(name="outp", bufs=2) as outp,
            tc.tile_pool(name="ps", bufs=1, space="PSUM") as ps,
        ):
            ones = const.tile([128, 128], F16)
            nc.vector.memset(ones, 1.0)

            wk_s = big.tile([128, KT, MW], F16, tag="wk_s")
            ct = big.tile([128, KT, NK], F16, tag="ct")
            wv_s = big.tile([128, KT, MW], F16, tag="wv_s")
            wq_s = big.tile([128, KT, MW], F16, tag="wq_s")
            xt = big.tile([128, KT, NQ], F16, tag="xt")
            wo_s = big.tile([128, HL, D], F16, tag="wo_s")
            qT = big.tile([128, HL, NQ], F16, tag="qT")
            kT = big.tile([128, HL, NK], F16, tag="kT")
            v = big.tile([128, JT, MW], F16, tag="v")
            attT = big.tile([128, HL, NQ], F16, tag="attT")

            # DMA issue order = arrival priority (512-col chunks).
            # K proj (wk+ct) first, wv early for V proj, then Q proj's wq+xt.
            for k in range(KT):
                nc.sync.dma_start(out=wk_s[:, k, :], in_=wk[k, :, :])
            for c in range(4):
                cs = slice(c * 512, (c + 1) * 512)
                for k in range(KT):
                    nc.sync.dma_start(out=ct[:, k, cs], in_=condT[k, :, cs])
                if c == 0:
                    for k in range(KT):
                        nc.sync.dma_start(out=wv_s[:, k, :], in_=wv[k, :, :])
            for k in range(KT):
                nc.sync.dma_start(out=wq_s[:, k, :], in_=wq[k, :, :])
            for c in range(4):
                cs = slice(c * 512, (c + 1) * 512)
                for k in range(KT):
                    nc.sync.dma_start(out=xt[:, k, cs], in_=xT[k, :, cs])
            for h in range(HL):
                nc.sync.dma_start(out=wo_s[:, h, :], in_=wo[h, :, :])

            # ---- projection chunk emitters ----
            def kq_chunk(dst, w_s, src, h, c, nm):
                # dst[:, h, c*512:(c+1)*512] = W_h.T @ src chunk  (contract D)
                acc = ps.tile([128, 512], F32, tag="pp", bufs=2,
                              name=f"acc{nm}_{h}_{c}")
                for k in range(KT):
                    nc.tensor.matmul(
                        acc,
                        w_s[:, k, h * DH:(h + 1) * DH],
                        src[:, k, c * 512:(c + 1) * 512],
                        start=(k == 0), stop=(k == KT - 1))
                nc.vector.tensor_copy(dst[:, h, c * 512:(c + 1) * 512], acc)

            def v_chunk(jt):
                # v[:, jt, :] = condT_jt.T @ Wv (all 4 heads wide)
                acc = ps.tile([128, MW], F32, tag="pp", bufs=2,
                              name=f"accv_{jt}")
                for k in range(KT):
                    nc.tensor.matmul(
                        acc,
                        ct[:, k, jt * 128:(jt + 1) * 128],
                        wv_s[:, k, :],
                        start=(k == 0), stop=(k == KT - 1))
                nc.vector.tensor_copy(v[:, jt, :], acc)

            # upfront: K(all heads, c-outer so each arriving ct chunk feeds 4
            # chunks of PE work), V(all), Q0. This is ~75us of dense PE work
            # that fully covers the ~40us input-DMA window; attention h0
            # starts right after with everything it needs resident.
            for c in range(4):
                for h in range(HL):
                    kq_chunk(kT, wk_s, ct, h, c, "k")
            for jt in range(JT):
                v_chunk(jt)
            for c in range(4):
                kq_chunk(qT, wq_s, xt, 0, c, "q")

            # Q projections for heads 1-3 interleave into attention slack
            pending = []
            for h in range(1, HL):
                for c in range(4):
                    pending.append((qT, wq_s, xt, h, c, "q"))
            pending.reverse()  # pop() from the front of the logical order

            # ---- O projection emitters ----
            # Accumulates all 4 heads in PSUM per [128,512] chunk. Early rows
            # (it 0-7, available once h3-half0 is done) run inside h3-half1's
            # jt loop on the then-free "pp" tag with DVE copies; the rest run
            # at the tail on the "sc" tag with copies on the then-idle Scalar.
            def oproj_it(it, tag, engine):
                if tag == "pp":
                    pos = [ps.tile([128, 512], F32, tag="pp", bufs=2,
                                   name=f"po_{it}_{nh}") for nh in range(2)]
                else:
                    po = ps.tile([128, 1024], F32, tag="sc", bufs=2,
                                 name=f"po_{it}")
                    pos = [po[:, 0:512], po[:, 512:1024]]
                for h in range(HL):
                    for nh in range(2):
                        nc.tensor.matmul(
                            pos[nh],
                            attT[:, h, it * 128:(it + 1) * 128],
                            wo_s[:, h, nh * 512:(nh + 1) * 512],
                            start=(h == 0), stop=(h == HL - 1))
                ob = outp.tile([128, 1024], F16, tag="ob", name=f"ob_{it}")
                for nh in range(2):
                    s5 = slice(nh * 512, (nh + 1) * 512)
                    if engine == "act":
                        nc.scalar.copy(ob[:, s5], pos[nh])
                    else:
                        nc.vector.tensor_copy(ob[:, s5], pos[nh])
                nc.sync.dma_start(
                    out=out[it * 128:(it + 1) * 128, :], in_=ob)

            # ---- attention per head (two 1024-query halves each) ----
            for h in range(HL):
                for half in range(2):
                    i0 = half * 1024
                    avs = [ps.tile([128, 512], F32, tag="av", bufs=2,
                                   name=f"av_{h}_{half}_{i}") for i in range(2)]
                    den_s = denp.tile([128, 1024], F16, tag="den_s",
                                      name=f"den_s_{h}_{half}")
                    for jt in range(JT):
                        sc = ps.tile([128, 1024], F32, tag="sc", bufs=2,
                                     name=f"sc_{h}_{half}_{jt}")
                        for ih in range(2):
                            nc.tensor.matmul(
                                sc[:, ih * 512:(ih + 1) * 512],
                                kT[:, h, jt * 128:(jt + 1) * 128],
                                qT[:, h, i0 + ih * 512:i0 + (ih + 1) * 512],
                                start=True, stop=True)
                        esc = expp.tile([128, 1024], F16, tag="esc",
                                        name=f"esc_{h}_{half}_{jt}")
                        nc.scalar.activation(esc, sc, EXP)
                        for ih in range(2):
                            nc.tensor.matmul(
                                avs[ih],
                                v[:, jt, h * DH:(h + 1) * DH],
                                esc[:, ih * 512:(ih + 1) * 512],
                                start=(jt == 0), stop=(jt == JT - 1))
                        with tc.high_priority():
                            if jt == 0:
                                nc.vector.tensor_copy(den_s, esc)
                            else:
                                nc.vector.tensor_add(den_s, den_s, esc)
                        # light projection interleave: 2 chunks per half
                        if pending and jt in (5, 13):
                            kq_chunk(*pending.pop())
                        # early O-proj rows inside h3's second half
                        if h == HL - 1 and half == 1 and jt % 2 == 1:
                            oproj_it(jt // 2, "pp", "dve")
                    den_bc = ps.tile([128, 1024], F32, tag="sc", bufs=2,
                                     name=f"den_bc_{h}_{half}")
                    den_rec = denp.tile([128, 1024], F32, tag="den_rec",
                                        name=f"den_rec_{h}_{half}")
                    with tc.high_priority():
                        for ih in range(2):
                            s5 = slice(ih * 512, (ih + 1) * 512)
                            nc.tensor.matmul(den_bc[:, s5], ones, den_s[:, s5],
                                             start=True, stop=True)
                        nc.vector.reciprocal_approx_fast(
                            out=den_rec, in_=den_bc)
                        for ih in range(2):
                            s5 = slice(ih * 512, (ih + 1) * 512)
                            nc.vector.tensor_mul(
                                attT[:, h, i0 + ih * 512:i0 + (ih + 1) * 512],
                                avs[ih], den_rec[:, s5])

            # remaining O-proj rows (it 8-15) at the tail
            for it in range(IT // 2, IT):
                oproj_it(it, "sc", "act")
    nc.finalize()
    return nc


_NC_CACHE = None


def _get_nc():
    global _NC_CACHE
    if _NC_CACHE is None:
        _NC_CACHE = build_nc()
    return _NC_CACHE


def make_in_maps(x, cond, Wq, Wk, Wv, Wo, bo):
    wq16 = (np.asarray(Wq, np.float32) * SCALE).astype(np.float16)
    wk16 = np.asarray(Wk, np.float32).astype(np.float16)
    wv16 = np.asarray(Wv, np.float32).astype(np.float16)
    wo16 = np.asarray(Wo, np.float32).astype(np.float16)
    x16 = np.asarray(x, np.float32).astype(np.float16)
    c16 = np.asarray(cond, np.float32).astype(np.float16)
    in_maps = []
    for c in range(NCORES):
        b, hh = c // 2, c % 2
        cols = slice(hh * MW, (hh + 1) * MW)
        in_maps.append({
            "xT": np.ascontiguousarray(x16[b].T).reshape(KT, 128, NQ),
            "condT": np.ascontiguousarray(c16[b].T).reshape(KT, 128, NK),
            "wq": np.ascontiguousarray(wq16[:, cols]).reshape(KT, 128, MW),
            "wk": np.ascontiguousarray(wk16[:, cols]).reshape(KT, 128, MW),
            "wv": np.ascontiguousarray(wv16[:, cols]).reshape(KT, 128, MW),
            "wo": np.ascontiguousarray(wo16[cols, :]).reshape(HL, 128, D),
        })
    return in_maps


def kernel(x, cond, Wq, Wk, Wv, Wo, bo, _trace=False, _trace_kwargs=None):
    nc = _get_nc()
    in_maps = make_in_maps(x, cond, Wq, Wk, Wv, Wo, bo)
    kw = {}
    if _trace:
        kw = {"trace": True, "trace_kwargs": _trace_kwargs or {}}
    res = run_bass_kernel_spmd(nc, in_maps, list(range(NCORES)), **kw)
    bo32 = np.asarray(bo, np.float32)
    out = np.empty((B, NQ, D), dtype=np.float32)
    for b in range(B):
        out[b] = (res.results[2 * b]["out"].astype(np.float32)
                  + res.results[2 * b + 1]["out"].astype(np.float32) + bo32)
    if _trace:
        return out, res
    return out


if __name__ == "__main__":
    rng = np.random.default_rng(0)
    s = 0.02
    x = rng.standard_normal((B, NQ, D), dtype=np.float32)
    cond = rng.standard_normal((B, NK, D), dtype=np.float32)
    Wq = (rng.standard_normal((D, D), dtype=np.float32) * s)
    Wk = (rng.standard_normal((D, D), dtype=np.float32) * s)
    Wv = (rng.standard_normal((D, D), dtype=np.float32) * s)
    Wo = (rng.standard_normal((D, D), dtype=np.float32) * s)
    bo = (rng.standard_normal((D,), dtype=np.float32) * s)

    def ref_np(x, cond):
        q = (x @ Wq).reshape(B, NQ, H, DH).transpose(0, 2, 1, 3)
        k = (cond @ Wk).reshape(B, NK, H, DH).transpose(0, 2, 1, 3)
        v = (cond @ Wv).reshape(B, NK, H, DH).transpose(0, 2, 1, 3)
        sim = np.einsum('bhid,bhjd->bhij', q, k) * SCALE
        sim = sim - sim.max(axis=-1, keepdims=True)
        a = np.exp(sim)
        a = a / a.sum(axis=-1, keepdims=True)
        o = np.einsum('bhij,bhjd->bhid', a, v)
        o = o.transpose(0, 2, 1, 3).reshape(B, NQ, D)
        return o @ Wo + bo

    import time
    t0 = time.time()
    got = kernel(x=x, cond=cond, Wq=Wq, Wk=Wk, Wv=Wv, Wo=Wo, bo=bo)
    print(f"kernel run {time.time()-t0:.1f}s")
    exp = ref_np(x.astype(np.float64), cond.astype(np.float64))
    err = np.abs(got - exp)
    rel = np.linalg.norm(got - exp) / np.linalg.norm(exp)
    print(f"rel_l2={rel:.3e} absmax_rel={err.max()/np.abs(exp).max():.3e}")


# revision 7
# speedup vs baseline: 1.0374x; 1.0374x over previous
"""CrossAttention TRN2 kernel (head-parallel, fp16 operands, host-summed partials).

Problem (hardcoded shapes):
  x    [4, 2048, 1024], cond [4, 2048, 1024]
  Wq/Wk/Wv [1024, 1024], Wo [1024, 1024], bo [1024]
  out = softmax((x@Wq) 8 heads of 128 @ (cond@Wk)^T * 0.125) @ (cond@Wv) @ Wo + bo

Sharding: 8 cores = (batch b in 0..3) x (head-half hh in 0..1).
Each core computes Q/K/V projections and attention for ITS 4 heads over the
full 2048 queries, then a PARTIAL output projection (contraction over its 512
inner columns only), written as fp16 [2048, 1024]. The host sums the two
partials per batch and adds the bias — no duplicated projection work and no
on-chip collectives. Total matmul rows/core: 532k -> PE-bound at ~0.5ns/row.

Schedule notes (v2 — PE-dense restructure):
  - O projection accumulates over heads directly in PSUM at the END of the
    kernel (4 matmuls per [128,512] output chunk) instead of per-head DVE
    adds into an SBUF accumulator. This removes ~128 DVE tensor_tensor ops
    (~100us of DVE time) that made Vector the attention-phase bottleneck.
    The PSUM->SBUF output copies run on the Scalar engine (idle at the tail).
  - K/Q projections for heads 1-3 are emitted inside head-0/1's attention
    jt-loops, so the Tile scheduler fills PE gaps (while Act does exp) with
    projection matmuls instead of idling.
  - Scores kept transposed [j, i]; softmax denominator accumulated on DVE in
    fp16 (high priority), partition-reduced via ones-matmul, inverted with a
    single 1024-wide reciprocal_approx_fast.
  - PSUM: sc 2x[128,1024] (4 banks) + av 2x[128,512] (2) + pp 2x[128,512]
    (2) = 8 banks. O-proj accumulators and den broadcast reuse the sc tag.
"""
import numpy as np

import concourse.bacc as bacc
import concourse.tile as tile
from concourse import mybir
from concourse.bass_utils import run_bass_kernel_spmd

F32 = mybir.dt.float32
F16 = mybir.dt.float16
EXP = mybir.ActivationFunctionType.Exp

B, NQ, NK, D = 4, 2048, 2048, 1024
H, DH = 8, 128
SCALE = 64 ** -0.5
NCORES = 8
KT = D // 128                         # contraction tiles (8)
HL = 4                                # heads per core
JT = NK // 128                        # key tiles (16)
IT = NQ // 128                        # output row tiles (16)
MW = HL * DH                          # 512 inner columns per core


def build_nc():
    nc = bacc.Bacc()
    xT = nc.declare_dram_parameter("xT", [KT, 128, NQ], F16, isOutput=False)
    condT = nc.declare_dram_parameter("condT", [KT, 128, NK], F16, isOutput=False)
    wq = nc.declare_dram_parameter("wq", [KT, 128, MW], F16, isOutput=False)
    wk = nc.declare_dram_parameter("wk", [KT, 128, MW], F16, isOutput=False)
    wv = nc.declare_dram_parameter("wv", [KT, 128, MW], F16, isOutput=False)
    wo = nc.declare_dram_parameter("wo", [HL, 128, D], F16, isOutput=False)
    out = nc.declare_dram_parameter("out", [NQ, D], F16, isOutput=True)

    with tile.TileContext(nc) as tc:
        with (
            nc.allow_low_precision(reason="fp16 matmul operands are intended"),
            tc.tile_pool(name="const", bufs=1) as const,
            tc.tile_pool(name="big", bufs=1) as big,
            tc.tile_pool(name="expp", bufs=6) as expp,
            tc.tile_pool(name="den", bufs=2) as denp,
            tc.tile_pool(name="outp", bufs=2) as outp,
            tc.tile_pool(name="ps", bufs=1, space="PSUM") as ps,
        ):
            ones = const.tile([128, 128], F16)
            nc.vector.memset(ones, 1.0)
            # preload the Exp activation table while the PE does projections
            # (the implicit ACT_TABLE_LOAD otherwise lands on the first real
            # exp, on the attention critical path)
            warm = const.tile([128, 1], F16)
            nc.scalar.activation(warm, ones[:, 0:1], EXP)

            wk_s = big.tile([128, KT, MW], F16, tag="wk_s")
            ct = big.tile([128, KT, NK], F16, tag="ct")
            wv_s = big.tile([128, KT, MW], F16, tag="wv_s")
            wq_s = big.tile([128, KT, MW], F16, tag="wq_s")
            xt = big.tile([128, KT, NQ], F16, tag="xt")
            wo_s = big.tile([128, HL, D], F16, tag="wo_s")
            qT = big.tile([128, HL, NQ], F16, tag="qT")
            kT = big.tile([128, HL, NK], F16, tag="kT")
            v = big.tile([128, JT, MW], F16, tag="v")
            attT = big.tile([128, HL, NQ], F16, tag="attT")

            # DMA issue order = arrival priority (512-col chunks).
            # K proj (wk+ct) first — interleaved per k-tile so the first
            # matmuls can start after ~0.5MB instead of ~2MB — then wv for
            # V proj, then Q proj's wq+xt.
            for k in range(KT):
                nc.sync.dma_start(out=wk_s[:, k, :], in_=wk[k, :, :])
                nc.sync.dma_start(out=ct[:, k, 0:512], in_=condT[k, :, 0:512])
            for c in range(1, 4):
                cs = slice(c * 512, (c + 1) * 512)
                for k in range(KT):
                    nc.sync.dma_start(out=ct[:, k, cs], in_=condT[k, :, cs])
                if c == 1:
                    for k in range(KT):
                        nc.sync.dma_start(out=wv_s[:, k, :], in_=wv[k, :, :])
            for k in range(KT):
                nc.sync.dma_start(out=wq_s[:, k, :], in_=wq[k, :, :])
            for c in range(4):
                cs = slice(c * 512, (c + 1) * 512)
                for k in range(KT):
                    nc.sync.dma_start(out=xt[:, k, cs], in_=xT[k, :, cs])
            for h in range(HL):
                nc.sync.dma_start(out=wo_s[:, h, :], in_=wo[h, :, :])

            # ---- projection chunk emitters ----
            def kq_chunk(dst, w_s, src, h, c, nm):
                # dst[:, h, c*512:(c+1)*512] = W_h.T @ src chunk  (contract D)
                acc = ps.tile([128, 512], F32, tag="pp", bufs=2,
                              name=f"acc{nm}_{h}_{c}")
                for k in range(KT):
                    nc.tensor.matmul(
                        acc,
                        w_s[:, k, h * DH:(h + 1) * DH],
                        src[:, k, c * 512:(c + 1) * 512],
                        start=(k == 0), stop=(k == KT - 1))
                nc.vector.tensor_copy(dst[:, h, c * 512:(c + 1) * 512], acc)

            def v_chunk(jt):
                # v[:, jt, :] = condT_jt.T @ Wv (all 4 heads wide)
                acc = ps.tile([128, MW], F32, tag="pp", bufs=2,
                              name=f"accv_{jt}")
                for k in range(KT):
                    nc.tensor.matmul(
                        acc,
                        ct[:, k, jt * 128:(jt + 1) * 128],
                        wv_s[:, k, :],
                        start=(k == 0), stop=(k == KT - 1))
                nc.vector.tensor_copy(v[:, jt, :], acc)

            # upfront: K(all heads, c-outer so each arriving ct chunk feeds 4
            # chunks of PE work), V(all), Q0. This is ~75us of dense PE work
            # that fully covers the ~40us input-DMA window; attention h0
            # starts right after with everything it needs resident.
            for c in range(4):
                for h in range(HL):
                    kq_chunk(kT, wk_s, ct, h, c, "k")
            for jt in range(JT):
                v_chunk(jt)
            for c in range(4):
                kq_chunk(qT, wq_s, xt, 0, c, "q")

            # Q projections for heads 1-3 interleave into attention slack
            pending = []
            for h in range(1, HL):
                for c in range(4):
                    pending.append((qT, wq_s, xt, h, c, "q"))
            pending.reverse()  # pop() from the front of the logical order

            # ---- O projection emitters ----
            # Accumulates all 4 heads in PSUM per [128,512] chunk. Early rows
            # (it 0-7, available once h3-half0 is done) run inside h3-half1's
            # jt loop on the then-free "pp" tag with DVE copies; the rest run
            # at the tail on the "sc" tag with copies on the then-idle Scalar.
            def oproj_it(it, tag, engine):
                if tag == "pp":
                    pos = [ps.tile([128, 512], F32, tag="pp", bufs=2,
                                   name=f"po_{it}_{nh}") for nh in range(2)]
                else:
                    po = ps.tile([128, 1024], F32, tag="sc", bufs=2,
                                 name=f"po_{it}")
                    pos = [po[:, 0:512], po[:, 512:1024]]
                for h in range(HL):
                    for nh in range(2):
                        nc.tensor.matmul(
                            pos[nh],
                            attT[:, h, it * 128:(it + 1) * 128],
                            wo_s[:, h, nh * 512:(nh + 1) * 512],
                            start=(h == 0), stop=(h == HL - 1))
                ob = outp.tile([128, 1024], F16, tag="ob", name=f"ob_{it}")
                for nh in range(2):
                    s5 = slice(nh * 512, (nh + 1) * 512)
                    if engine == "act":
                        nc.scalar.copy(ob[:, s5], pos[nh])
                    else:
                        nc.vector.tensor_copy(ob[:, s5], pos[nh])
                nc.sync.dma_start(
                    out=out[it * 128:(it + 1) * 128, :], in_=ob)

            # ---- attention per head (two 1024-query halves each) ----
            for h in range(HL):
                for half in range(2):
                    i0 = half * 1024
                    avs = [ps.tile([128, 512], F32, tag="av", bufs=2,
                                   name=f"av_{h}_{half}_{i}") for i in range(2)]
                    den_s = denp.tile([128, 1024], F16, tag="den_s",
                                      name=f"den_s_{h}_{half}")
                    for jt in range(JT):
                        sc = ps.tile([128, 1024], F32, tag="sc", bufs=2,
                                     name=f"sc_{h}_{half}_{jt}")
                        for ih in range(2):
                            nc.tensor.matmul(
                                sc[:, ih * 512:(ih + 1) * 512],
                                kT[:, h, jt * 128:(jt + 1) * 128],
                                qT[:, h, i0 + ih * 512:i0 + (ih + 1) * 512],
                                start=True, stop=True)
                        esc = expp.tile([128, 1024], F16, tag="esc",
                                        name=f"esc_{h}_{half}_{jt}")
                        nc.scalar.activation(esc, sc, EXP)
                        for ih in range(2):
                            nc.tensor.matmul(
                                avs[ih],
                                v[:, jt, h * DH:(h + 1) * DH],
                                esc[:, ih * 512:(ih + 1) * 512],
                                start=(jt == 0), stop=(jt == JT - 1))
                        with tc.high_priority():
                            if jt == 0:
                                nc.vector.tensor_copy(den_s, esc)
                            else:
                                nc.vector.tensor_add(den_s, den_s, esc)
                        # light projection interleave: 2 chunks per half
                        if pending and jt in (5, 13):
                            kq_chunk(*pending.pop())
                        # early O-proj rows inside h3's second half
                        if h == HL - 1 and half == 1 and jt % 2 == 1:
                            oproj_it(jt // 2, "pp", "dve")
                    # den broadcast on the "pp" tag — keeping it off the "sc"
                    # tag decouples the half boundary from the score-tile
                    # rotation (slot WAR would serialize sc15 -> den -> sc0').
                    den_rec = denp.tile([128, 1024], F32, tag="den_rec",
                                        name=f"den_rec_{h}_{half}")
                    with tc.high_priority():
                        for ih in range(2):
                            s5 = slice(ih * 512, (ih + 1) * 512)
                            den_bc = ps.tile([128, 512], F32, tag="pp",
                                             bufs=2,
                                             name=f"den_bc_{h}_{half}_{ih}")
                            nc.tensor.matmul(den_bc, ones, den_s[:, s5],
                                             start=True, stop=True)
                            nc.vector.reciprocal_approx_fast(
                                out=den_rec[:, s5], in_=den_bc)
                            nc.vector.tensor_mul(
                                attT[:, h, i0 + ih * 512:i0 + (ih + 1) * 512],
                                avs[ih], den_rec[:, s5])

            # remaining O-proj rows (it 8-15) at the tail
            for it in range(IT // 2, IT):
                oproj_it(it, "sc", "act")
    nc.finalize()
    return nc


_NC_CACHE = None


def _get_nc():
    global _NC_CACHE
    if _NC_CACHE is None:
        _NC_CACHE = build_nc()
    return _NC_CACHE


def make_in_maps(x, cond, Wq, Wk, Wv, Wo, bo):
    wq16 = (np.asarray(Wq, np.float32) * SCALE).astype(np.float16)
    wk16 = np.asarray(Wk, np.float32).astype(np.float16)
    wv16 = np.asarray(Wv, np.float32).astype(np.float16)
    wo16 = np.asarray(Wo, np.float32).astype(np.float16)
    x16 = np.asarray(x, np.float32).astype(np.float16)
    c16 = np.asarray(cond, np.float32).astype(np.float16)
    in_maps = []
    for c in range(NCORES):
        b, hh = c // 2, c % 2
        cols = slice(hh * MW, (hh + 1) * MW)
        in_maps.append({
            "xT": np.ascontiguousarray(x16[b].T).reshape(KT, 128, NQ),
            "condT": np.ascontiguousarray(c16[b].T).reshape(KT, 128, NK),
            "wq": np.ascontiguousarray(wq16[:, cols]).reshape(KT, 128, MW),
            "wk": np.ascontiguousarray(wk16[:, cols]).reshape(KT, 128, MW),
            "wv": np.ascontiguousarray(wv16[:, cols]).reshape(KT, 128, MW),
            "wo": np.ascontiguousarray(wo16[cols, :]).reshape(HL, 128, D),
        })
    return in_maps


def kernel(x, cond, Wq, Wk, Wv, Wo, bo, _trace=False, _trace_kwargs=None):
    nc = _get_nc()
    in_maps = make_in_maps(x, cond, Wq, Wk, Wv, Wo, bo)
    kw = {}
    if _trace:
        kw = {"trace": True, "trace_kwargs": _trace_kwargs or {}}
    res = run_bass_kernel_spmd(nc, in_maps, list(range(NCORES)), **kw)
    bo32 = np.asarray(bo, np.float32)
    out = np.empty((B, NQ, D), dtype=np.float32)
    for b in range(B):
        out[b] = (res.results[2 * b]["out"].astype(np.float32)
                  + res.results[2 * b + 1]["out"].astype(np.float32) + bo32)
    if _trace:
        return out, res
    return out


if __name__ == "__main__":
    rng = np.random.default_rng(0)
    s = 0.02
    x = rng.standard_normal((B, NQ, D), dtype=np.float32)
    cond = rng.standard_normal((B, NK, D), dtype=np.float32)
    Wq = (rng.standard_normal((D, D), dtype=np.float32) * s)
    Wk = (rng.standard_normal((D, D), dtype=np.float32) * s)
    Wv = (rng.standard_normal((D, D), dtype=np.float32) * s)
    Wo = (rng.standard_normal((D, D), dtype=np.float32) * s)
    bo = (rng.standard_normal((D,), dtype=np.float32) * s)

    def ref_np(x, cond):
        q = (x @ Wq).reshape(B, NQ, H, DH).transpose(0, 2, 1, 3)
        k = (cond @ Wk).reshape(B, NK, H, DH).transpose(0, 2, 1, 3)
        v = (cond @ Wv).reshape(B, NK, H, DH).transpose(0, 2, 1, 3)
        sim = np.einsum('bhid,bhjd->bhij', q, k) * SCALE
        sim = sim - sim.max(axis=-1, keepdims=True)
        a = np.exp(sim)
        a = a / a.sum(axis=-1, keepdims=True)
        o = np.einsum('bhij,bhjd->bhid', a, v)
        o = o.transpose(0, 2, 1, 3).reshape(B, NQ, D)
        return o @ Wo + bo

    import time
    t0 = time.time()
    got = kernel(x=x, cond=cond, Wq=Wq, Wk=Wk, Wv=Wv, Wo=Wo, bo=bo)
    print(f"kernel run {time.time()-t0:.1f}s")
    exp = ref_np(x.astype(np.float64), cond.astype(np.float64))
    err = np.abs(got - exp)
    rel = np.linalg.norm(got - exp) / np.linalg.norm(exp)
    print(f"rel_l2={rel:.3e} absmax_rel={err.max()/np.abs(exp).max():.3e}")
